# revision 31
# baseline (speedup 1.0000x reference)
"""Trainium2 Bass kernel for nn_Block_31722628448427 (dense transformer block
with multi-head latent attention + gated FFN).

Sharding over 8 NeuronCores: core c handles batch b = c//2.  The attention
part (LN1, k/v projections, latent attention, out-projection) is computed for
the full batch on both cores of a pair; the latent->seq projection, residual,
LN2 and the FFN are computed for sequence half c%2 only.  No cross-core
communication.

Precision plan: x is staged bf16; LN statistics, softmax normalization and
residuals are fp32.  The k/v projections and the w3 (down) matmul run in
fp8e4 with DoubleRow (2 K-subtiles per pass), weights pre-scaled by WS host
side and descaled on the PSUM->SBUF copy.  Softmax weights are quantized to
fp8 at x256 and descaled after the out-projection.  w1/w2 are bf16 (fp8
optional via W12_FP8).  v lives entirely in SBUF (fp8) — no DRAM round-trip.
"""
import contextlib

import numpy as np
import ml_dtypes

import bass_rust
import concourse.bass as bass
import concourse.tile as tile
from concourse import mybir
from concourse.masks import make_identity

BF16 = ml_dtypes.bfloat16
F8NP = ml_dtypes.float8_e4m3
F32 = mybir.dt.float32
BF = mybir.dt.bfloat16
F8 = mybir.dt.float8e4
AF = mybir.ActivationFunctionType
OP = mybir.AluOpType
DR = mybir.MatmulPerfMode.DoubleRow

B, S, E, H, HD, LD, NL = 4, 4096, 1024, 16, 64, 256, 64
EPS = 1e-5
P = 128
NCORES = 8
WS = 32.0      # fp8 weight pre-scale
WSC = 256.0    # fp8 softmax-weight pre-scale
W12_FP8 = True   # w1 in fp8 DoubleRow; w2 stays bf16 (error headroom)


# --------------------------------------------------------------------------
# walrus on this toolchain accepts at most ONE embedded sync-wait per
# instruction; Tile emits more at cross-engine joins.  Spill extras onto
# standalone same-engine NoOps placed immediately before the instruction.
def _spill_extra_waits(nc):
    counter = 0
    for f in nc.m.functions:
        for bb in f.blocks:
            new_list = []
            changed = False
            for inst in bb.instructions:
                si = inst.sync_info
                waits = list(si.on_wait) if si is not None else []
                if len(waits) > 1:
                    for w in waits[:-1]:
                        nop = mybir.InstNoOp(name=f"wspill_{counter}", ins=[], outs=[])
                        counter += 1
                        nop.engine = inst.engine
                        nop.sync_info = bass_rust.SyncInfo(on_wait=[w], on_update=[])
                        new_list.append(nop)
                    inst.sync_info = bass_rust.SyncInfo(
                        on_wait=waits[-1:], on_update=list(si.on_update)
                    )
                    changed = True
                new_list.append(inst)
            if changed:
                bb.instructions = new_list


# --------------------------------------------------------------------------
def build_program(nc, seq=S, silu_via_sigmoid=False, phases="ABC"):
    """Emit the per-core program.  `seq` lets tests build a smaller version.
    `silu_via_sigmoid` replaces the Silu LUT (not implemented in CoreSim)
    with Sigmoid + multiply; used only by the simulator test.
    `phases` truncates the program for debugging ("A", "AB", "ABC")."""
    ST = seq // P            # token tiles over the full sequence
    NSC = seq // 512         # 512-wide chunks over the full sequence
    HSEQ = seq // 2          # this core's token count for the FFN part
    NCH = max(HSEQ // 512, 1)   # FFN chunks
    CH = HSEQ // NCH         # tokens per FFN chunk (512)
    CT = CH // P             # token tiles per chunk (4)
    HP = H // 2              # head pairs
    ET = E // P              # 8 e-tiles
    FT = 4 * E // P          # 32 hidden tiles
    W1DT = F8 if W12_FP8 else BF

    dram = lambda name, shape, dt, kind="ExternalInput": nc.dram_tensor(
        name, shape, dt, kind=kind
    )
    x_full_d = dram("x_full", [seq, E], BF)
    x_half_d = dram("x_half", [HSEQ, E], BF)
    wk_d = dram("wk", [P, ET, H * HD], F8)
    wv_d = dram("wv", [P, ET, H * HD], F8)
    wq_d = dram("wq", [P, H * (LD // P) * HD], BF)
    lat_d = dram("lat", [P, H * (LD // P) * NL], BF)
    wo_d = dram("wo", [HD, H, E], BF)
    wproj_d = dram("wproj", [NL, HSEQ], BF)
    w1_d = dram("w1", [FT, P, ET, P], W1DT)
    w2_d = dram("w2", [FT, P, ET, P], BF)
    w3_d = dram("w3", [P, FT, E], F8)
    out_d = dram("out", [HSEQ, E], BF, kind="ExternalOutput")

    with tile.TileContext(nc) as tc, contextlib.ExitStack() as ctx:
        const = ctx.enter_context(tc.tile_pool(name="const", bufs=1))
        longp = ctx.enter_context(tc.tile_pool(name="longp", bufs=1))
        psA = ctx.enter_context(tc.tile_pool(name="psA", bufs=4, space="PSUM"))
        psB = ctx.enter_context(tc.tile_pool(name="psB", bufs=1, space="PSUM"))
        psT = ctx.enter_context(tc.tile_pool(name="psT", bufs=2, space="PSUM"))

        ident = const.tile([P, P], BF)
        make_identity(nc, ident)
        ident8 = const.tile([P, P], F8)
        make_identity(nc, ident8)
        eps_t = const.tile([P, 1], F32)
        nc.vector.memset(eps_t, EPS)

        wproj_sb = longp.tile([NL, HSEQ], BF)
        nc.sync.dma_start(out=wproj_sb, in_=wproj_d.ap())
        attn_sb = longp.tile([NL, E], BF)
        # qT pair-stacked: partitions j*HD:(j+1)*HD hold head 2*hp+j's q.T
        qT_sb = longp.tile([P, (H // 2) * NL], BF)

        # kT (bf16) and v (fp8) live fully in SBUF across phases A and B
        kT_ctx = contextlib.ExitStack()
        kT_p = kT_ctx.enter_context(tc.tile_pool(name="kTp", bufs=1))
        kT_sb = kT_p.tile([P, HP, seq], BF)
        v_ctx = contextlib.ExitStack()
        v_p = v_ctx.enter_context(tc.tile_pool(name="vp", bufs=1))
        v_sb = v_p.tile([P, ST, H * HD], F8)

        # ---------------- phase A: LN1, q/k/v projections ----------------
        with contextlib.ExitStack() as actx:
            xst = actx.enter_context(tc.tile_pool(name="xst", bufs=3))

            def emit_x_load(sc):
                x4_t = xst.tile([P, 4, E], BF, tag="x")
                nc.sync.dma_start(
                    out=x4_t,
                    in_=x_full_d.ap()[sc * 512:(sc + 1) * 512, :].rearrange(
                        "(t p) e -> p t e", p=P),
                )
                return x4_t

            # chunk-0 tokens first in the DMA queue so LN1 starts while the
            # qT matmuls below still run
            x_pre = emit_x_load(0)

            # qT in a short-lived innermost pool (wq/lat freed after)
            small_ctx = contextlib.ExitStack()
            small = small_ctx.enter_context(tc.tile_pool(name="small", bufs=1))
            wq_sb = small.tile([P, H * (LD // P) * HD], BF)
            lat_sb = small.tile([P, H * (LD // P) * NL], BF)
            qn = H * (LD // P) * HD
            for q2 in range(2):
                nc.sync.dma_start(
                    out=wq_sb[:, q2 * qn // 2:(q2 + 1) * qn // 2],
                    in_=wq_d.ap()[:, q2 * qn // 2:(q2 + 1) * qn // 2])
                nc.sync.dma_start(
                    out=lat_sb[:, q2 * qn // 2:(q2 + 1) * qn // 2],
                    in_=lat_d.ap()[:, q2 * qn // 2:(q2 + 1) * qn // 2])

            # qT[h] = Wq[h].T @ latT[h] -> [HD, NL], pair-stacked on partitions.
            # PE outputs must stay at the operands' base partition (0), so odd
            # heads reach partitions 64..127 via one batched SBUF->SBUF DMA.
            q_odd = small.tile([HD, HP * NL], BF)
            for h in range(H):
                hp, j = h // 2, h % 2
                ps_q = psA.tile([HD, NL], F32, tag="ps")
                for kt in range(LD // P):
                    iq = (h * (LD // P) + kt) * HD
                    il = (h * (LD // P) + kt) * NL
                    nc.tensor.matmul(
                        ps_q,
                        wq_sb[:, iq:iq + HD],
                        lat_sb[:, il:il + NL],
                        start=(kt == 0), stop=(kt == LD // P - 1),
                    )
                if j == 0:
                    nc.vector.tensor_copy(
                        qT_sb[0:HD, hp * NL:(hp + 1) * NL], ps_q
                    )
                else:
                    nc.vector.tensor_copy(
                        q_odd[:, hp * NL:(hp + 1) * NL], ps_q
                    )
            nc.sync.dma_start(out=qT_sb[HD:P, :], in_=q_odd)
            small_ctx.close()

            stat = actx.enter_context(tc.tile_pool(name="stat", bufs=4))
            xnT_p = actx.enter_context(tc.tile_pool(name="xnT", bufs=2))

            wv_ctx = contextlib.ExitStack()
            wv_p = wv_ctx.enter_context(tc.tile_pool(name="wv", bufs=1))
            wv_sb = wv_p.tile([P, ET, H * HD], F8)
            wk_sb = wv_p.tile([P, ET, H * HD], F8)
            nc.sync.dma_start(out=wv_sb, in_=wv_d.ap())
            nc.sync.dma_start(out=wk_sb, in_=wk_d.ap())

            # LN1 + transpose + v-projection per token tile; k-projection per
            # 512-token chunk.  xnT lives only per-chunk (fp8).
            for sc in range(NSC):
                xnT_c = xnT_p.tile([P, ET, 512], F8, tag="xnTc")
                x4_t = x_pre
                if sc + 1 < NSC:
                    x_pre = emit_x_load(sc + 1)
                for t4 in range(4):
                    tt = sc * 4 + t4
                    x_t = x4_t[:, t4, :]
                    st_t = stat.tile([P, 2, 6], F32, tag="st")
                    xg = x_t.rearrange("p (g d) -> p g d", g=2)
                    for g in range(2):
                        nc.vector.bn_stats(out=st_t[:, g, :], in_=xg[:, g, :])
                    mv = stat.tile([P, 2], F32, tag="mv")
                    nc.vector.bn_aggr(out=mv, in_=st_t)
                    std = stat.tile([P, 1], F32, tag="std")
                    nc.scalar.activation(std, mv[:, 1:2], AF.Sqrt, bias=eps_t,
                                         scale=1.0)
                    rstd = stat.tile([P, 1], F32, tag="rstd")
                    nc.vector.reciprocal(rstd, std)
                    # normalize straight to fp8 (on the Pool engine), transpose
                    # in fp8, and move PSUM->SBUF with one wide strided copy
                    xn_t = xst.tile([P, E], F8, tag="xn")
                    nc.gpsimd.tensor_scalar(
                        out=xn_t, in0=x_t, scalar1=mv[:, 0:1], scalar2=rstd,
                        op0=OP.subtract, op1=OP.mult,
                    )
                    # fp8 PE transpose writes PSUM with element step 2
                    ps_t = psT.tile([P, ET * P, 2], F8, tag="pst")
                    ps_tv = ps_t[:, :, 0].rearrange("p (e q) -> p e q", q=P)
                    for et in range(ET):
                        nc.tensor.transpose(
                            ps_tv[:, et, :],
                            xn_t[:, et * P:(et + 1) * P], ident8,
                        )
                    nc.vector.tensor_copy(
                        xnT_c[:, :, t4 * P:(t4 + 1) * P], ps_tv,
                    )
                    # v projection for this token tile (DoubleRow fp8)
                    for n2 in range(2):
                        ps_v = psA.tile([P, 512], F32, tag="ps")
                        for e2 in range(ET // 2):
                            nc.tensor.matmul(
                                ps_v,
                                xnT_c[:, 2 * e2:2 * e2 + 2, t4 * P:(t4 + 1) * P],
                                wv_sb[:, 2 * e2:2 * e2 + 2,
                                      n2 * 512:(n2 + 1) * 512],
                                start=(e2 == 0), stop=(e2 == ET // 2 - 1),
                                perf_mode=DR,
                            )
                        nc.scalar.activation(
                            v_sb[:, tt, n2 * 512:(n2 + 1) * 512], ps_v,
                            AF.Copy, scale=1.0 / WS,
                        )
                # kT projection for this chunk (DoubleRow fp8)
                for hp in range(HP):
                    ps_k = psA.tile([P, 512], F32, tag="ps")
                    for e2 in range(ET // 2):
                        nc.tensor.matmul(
                            ps_k,
                            wk_sb[:, 2 * e2:2 * e2 + 2, hp * P:(hp + 1) * P],
                            xnT_c[:, 2 * e2:2 * e2 + 2, :],
                            start=(e2 == 0), stop=(e2 == ET // 2 - 1),
                            perf_mode=DR,
                        )
                    nc.scalar.activation(
                        kT_sb[:, hp, sc * 512:(sc + 1) * 512], ps_k,
                        AF.Copy, scale=1.0 / WS,
                    )
            wv_ctx.close()

        if phases == "A":
            v_ctx.close()
            kT_ctx.close()
            with tc.tile_pool(name="dbg", bufs=3) as dbg:
                for tt in range(HSEQ // P):
                    d_t = dbg.tile([P, E], BF)
                    nc.sync.dma_start(out=d_t, in_=x_half_d.ap()[tt * P:(tt + 1) * P, :])
                    nc.sync.dma_start(out=out_d.ap()[tt * P:(tt + 1) * P, :], in_=d_t)
            return nc

        # ---------------- phase B: latent attention ----------------
        # Softmax weights are stored as raw f8(exp(s)) (scores are small, so
        # exp() sits comfortably in fp8e4 range); the 1/z normalization is
        # deferred to the per-head out-projection, where latents live on
        # partitions and a per-partition scalar multiply works.
        with contextlib.ExitStack() as bctx:
            att = bctx.enter_context(tc.tile_pool(name="att", bufs=3))
            att2 = bctx.enter_context(tc.tile_pool(name="att2", bufs=2))
            qo = bctx.enter_context(tc.tile_pool(name="qo", bufs=1))
            # oT pair-stacked like qT; holds unnormalized sum(exp*v)
            oT_sb = qo.tile([P, HP * NL], BF)
            rz_all = qo.tile([P, HP], F32)

            def emit_scores_exp(hp):
                w_t = att.tile([P, seq], F8, tag="w")
                zp = att.tile([P, NSC], F32, tag="zp")
                for sc in range(NSC):
                    ps_s = psA.tile([P, 512], F32, tag="ps")
                    for j in range(2):
                        nc.tensor.matmul(
                            ps_s[j * NL:(j + 1) * NL, :],
                            qT_sb[j * HD:(j + 1) * HD, hp * NL:(hp + 1) * NL],
                            kT_sb[j * HD:(j + 1) * HD, hp, sc * 512:(sc + 1) * 512],
                            start=True, stop=True,
                        )
                    nc.scalar.activation(
                        w_t[:, sc * 512:(sc + 1) * 512], ps_s, AF.Exp,
                        scale=float(HD) ** -0.5,
                        accum_out=zp[:, sc:sc + 1],
                    )
                return w_t, zp

            cur = emit_scores_exp(0)
            for hp in range(HP):
                w_t, zp = cur
                nxt = emit_scores_exp(hp + 1) if hp + 1 < HP else None
                z_t = att.tile([P, 1], F32, tag="z")
                nc.vector.tensor_reduce(z_t, zp, axis=mybir.AxisListType.X, op=OP.add)
                nc.vector.reciprocal(rz_all[:, hp:hp + 1], z_t)
                wT_t = att2.tile([P, ST, P], F8, tag="wT")
                for s8 in range(ST // 8):
                    ps_w = psT.tile([P, 8 * P, 2], F8, tag="pst")
                    ps_wv = ps_w[:, :, 0].rearrange("p (s q) -> p s q", q=P)
                    for j in range(8):
                        st_i = s8 * 8 + j
                        nc.tensor.transpose(
                            ps_wv[:, j, :],
                            w_t[:, st_i * P:(st_i + 1) * P], ident8,
                        )
                    nc.vector.tensor_copy(
                        wT_t[:, s8 * 8:(s8 + 1) * 8, :], ps_wv,
                    )
                ps_o = psA.tile([P, P], F32, tag="ps")
                for s2 in range(ST // 2):
                    nc.tensor.matmul(
                        ps_o,
                        v_sb[:, 2 * s2:2 * s2 + 2, hp * P:(hp + 1) * P],
                        wT_t[:, 2 * s2:2 * s2 + 2, :],
                        start=(s2 == 0), stop=(s2 == ST // 2 - 1),
                        perf_mode=DR,
                    )
                for j in range(2):
                    nc.vector.tensor_copy(
                        oT_sb[j * HD:(j + 1) * HD, hp * NL:(hp + 1) * NL],
                        ps_o[j * HD:(j + 1) * HD, j * NL:(j + 1) * NL],
                    )
                cur = nxt

            # flatten oT to base partition 0 (odd heads via one batched
            # SBUF->SBUF DMA with a strided destination); same shift for the
            # odd heads' 1/z scalars
            oT_flat = qo.tile([HD, H, NL], BF)
            rz_even = rz_all[0:NL, :]
            rz_odd = qo.tile([NL, HP], F32)
            for hp in range(HP):
                nc.vector.tensor_copy(
                    oT_flat[:, 2 * hp, :],
                    oT_sb[0:HD, hp * NL:(hp + 1) * NL],
                )
            nc.sync.dma_start(
                out=oT_flat.rearrange("p (h2 j) l -> p h2 j l", j=2)[:, :, 1, :],
                in_=oT_sb[HD:P, :].rearrange("p (h l) -> p h l", l=NL),
            )
            nc.sync.dma_start(out=rz_odd, in_=rz_all[NL:P, :])

            # out-projection per head with deferred softmax normalization:
            # attn = sum_h (1/z_h) * (oT_h.T @ Wo[h])
            wo_p = bctx.enter_context(tc.tile_pool(name="wo", bufs=1))
            wo_sb = wo_p.tile([HD, H, E], BF)
            nc.sync.dma_start(out=wo_sb, in_=wo_d.ap())
            attn_acc = qo.tile([NL, E], F32)
            for h in range(H):
                hp, j = h // 2, h % 2
                rz_h = (rz_even if j == 0 else rz_odd)[:, hp:hp + 1]
                ps_h = psB.tile([NL, E], F32, tag="ps2")
                for n2 in range(2):
                    nc.tensor.matmul(
                        ps_h[:, n2 * 512:(n2 + 1) * 512],
                        oT_flat[:, h, :],
                        wo_sb[:, h, n2 * 512:(n2 + 1) * 512],
                        start=True, stop=True,
                    )
                if h == 0:
                    nc.vector.tensor_scalar_mul(attn_acc, ps_h, rz_h)
                else:
                    nc.vector.scalar_tensor_tensor(
                        out=attn_acc, in0=ps_h, scalar=rz_h, in1=attn_acc,
                        op0=OP.mult, op1=OP.add,
                    )
            nc.vector.tensor_copy(attn_sb, attn_acc)
        v_ctx.close()
        kT_ctx.close()

        if phases == "AB":
            with tc.tile_pool(name="dbg", bufs=3) as dbg:
                a_t = dbg.tile([NL, E], BF)
                nc.vector.tensor_copy(a_t, attn_sb)
                nc.sync.dma_start(out=out_d.ap()[0:NL, :], in_=a_t)
                for tt in range(HSEQ // P):
                    d_t = dbg.tile([P, E], BF, tag="d")
                    nc.sync.dma_start(out=d_t, in_=x_half_d.ap()[tt * P:(tt + 1) * P, :])
                    nc.sync.dma_start(
                        out=out_d.ap()[tt * P:(tt + 1) * P, :] if tt > 0
                        else out_d.ap()[NL:P, :], in_=d_t if tt > 0 else d_t[NL:P, :]
                    )
            return nc

        # ---------------- phase C: latent->seq, LN2, FFN ----------------
        with contextlib.ExitStack() as fctx:
            w3_p = fctx.enter_context(tc.tile_pool(name="w3", bufs=1))
            ffs = fctx.enter_context(tc.tile_pool(name="ffs", bufs=1))
            ff2 = fctx.enter_context(tc.tile_pool(name="ff2", bufs=2))
            wstream = fctx.enter_context(tc.tile_pool(name="wstream", bufs=4))
            xhs = fctx.enter_context(tc.tile_pool(name="xhs", bufs=2))
            stat2 = fctx.enter_context(tc.tile_pool(name="stat2", bufs=4))
            outs = fctx.enter_context(tc.tile_pool(name="outs", bufs=3))
            sw = fctx.enter_context(tc.tile_pool(name="sw", bufs=3))

            w3_sb = w3_p.tile([P, FT, E], F8)
            for k4 in range(4):
                nc.sync.dma_start(out=w3_sb[:, k4 * FT // 4:(k4 + 1) * FT // 4, :],
                                  in_=w3_d.ap()[:, k4 * FT // 4:(k4 + 1) * FT // 4, :])

            def emit_prologue(chk):
                """latent->seq + residual + LN2 + transpose for one chunk."""
                xres = ff2.tile([P, CT, E], F32, tag="xres")
                h2T = ff2.tile([P, ET, CH], BF, tag="h2T")
                if W12_FP8:
                    h2T8 = ff2.tile([P, ET, CH], F8, tag="h2T8")
                else:
                    h2T8 = h2T
                for t4 in range(CT):
                    s0 = chk * CH + t4 * P
                    ps_sq = psB.tile([P, E], F32, tag="ps2")
                    for n2 in range(2):
                        nc.tensor.matmul(
                            ps_sq[:, n2 * 512:(n2 + 1) * 512],
                            wproj_sb[:, s0:s0 + P],
                            attn_sb[:, n2 * 512:(n2 + 1) * 512],
                            start=True, stop=True,
                        )
                    xh_t = xhs.tile([P, E], BF, tag="xh")
                    nc.sync.dma_start(out=xh_t, in_=x_half_d.ap()[s0:s0 + P, :])
                    nc.vector.tensor_add(xres[:, t4, :], ps_sq, xh_t)
                    st2 = stat2.tile([P, 2, 6], F32, tag="st2")
                    xg2 = xres[:, t4, :].rearrange("p (g d) -> p g d", g=2)
                    for g in range(2):
                        nc.vector.bn_stats(out=st2[:, g, :], in_=xg2[:, g, :])
                    mv2 = stat2.tile([P, 2], F32, tag="mv2")
                    nc.vector.bn_aggr(out=mv2, in_=st2)
                    std2 = stat2.tile([P, 1], F32, tag="std2")
                    nc.scalar.activation(std2, mv2[:, 1:2], AF.Sqrt, bias=eps_t,
                                         scale=1.0)
                    rstd2 = stat2.tile([P, 1], F32, tag="rstd2")
                    nc.vector.reciprocal(rstd2, std2)
                    xn2_t = xhs.tile([P, E], BF, tag="xn2")
                    nc.gpsimd.tensor_scalar(
                        out=xn2_t, in0=xres[:, t4, :], scalar1=mv2[:, 0:1],
                        scalar2=rstd2, op0=OP.subtract, op1=OP.mult,
                    )
                    ps_t2 = psT.tile([P, ET * P], BF, tag="pst")
                    ps_t2v = ps_t2.rearrange("p (e q) -> p e q", q=P)
                    for et in range(ET):
                        nc.tensor.transpose(
                            ps_t2v[:, et, :],
                            xn2_t[:, et * P:(et + 1) * P], ident,
                        )
                    nc.vector.tensor_copy(
                        h2T[:, :, t4 * P:(t4 + 1) * P], ps_t2v,
                    )
                    if W12_FP8:
                        nc.scalar.activation(
                            h2T8[:, :, t4 * P:(t4 + 1) * P], ps_t2v, AF.Copy,
                        )
                return xres, h2T, h2T8

            def emit_ffn_matmul(ps, w_t, h2T_, dr):
                if dr:
                    for e2 in range(ET // 2):
                        nc.tensor.matmul(
                            ps, w_t[:, 2 * e2:2 * e2 + 2, :],
                            h2T_[:, 2 * e2:2 * e2 + 2, :],
                            start=(e2 == 0), stop=(e2 == ET // 2 - 1),
                            perf_mode=DR,
                        )
                else:
                    for et in range(ET):
                        nc.tensor.matmul(
                            ps, w_t[:, et, :], h2T_[:, et, :],
                            start=(et == 0), stop=(et == ET - 1),
                        )

            silu_scale = 1.0 / WS if W12_FP8 else 1.0

            def emit_gsw(ps_a, ps_g, gsw_slice, sw_t):
                if silu_via_sigmoid:
                    sg_t = sw.tile([P, CH], BF, tag="sgt")
                    nc.scalar.activation(sg_t, ps_a, AF.Sigmoid, scale=silu_scale)
                    a_t = sw.tile([P, CH], BF, tag="sat")
                    nc.scalar.activation(a_t, ps_a, AF.Copy, scale=silu_scale)
                    nc.vector.tensor_mul(sw_t, a_t, sg_t)
                else:
                    nc.scalar.activation(sw_t, ps_a, AF.Silu, scale=silu_scale)
                nc.vector.tensor_mul(gsw_slice, ps_g, sw_t)

            pro = emit_prologue(0)
            for chk in range(NCH):
                xres, h2T, h2T8 = pro
                gsw = ffs.tile([P, FT, CH], F8, tag="gsw")
                for m2 in range(FT // 2):
                    w1_t = wstream.tile([P, 2, ET, P], W1DT, tag="w1t")
                    nc.sync.dma_start(
                        out=w1_t,
                        in_=w1_d.ap()[2 * m2:2 * m2 + 2].rearrange(
                            "m p e q -> p m e q"))
                    w2_t = wstream.tile([P, 2, ET, P], BF, tag="w2t")
                    nc.sync.dma_start(
                        out=w2_t,
                        in_=w2_d.ap()[2 * m2:2 * m2 + 2].rearrange(
                            "m p e q -> p m e q"))
                    for i in range(2):
                        mt = 2 * m2 + i
                        ps_a = psA.tile([P, CH], F32, tag="ps")
                        emit_ffn_matmul(ps_a, w1_t[:, i], h2T8, W12_FP8)
                        sw_t = sw.tile([P, CH], BF, tag="swt")
                        ps_g = psA.tile([P, CH], F32, tag="ps")
                        emit_ffn_matmul(ps_g, w2_t[:, i], h2T, False)
                        emit_gsw(ps_a, ps_g, gsw[:, mt, :], sw_t)

                if chk + 1 < NCH:
                    pro = emit_prologue(chk + 1)

                for t4 in range(CT):
                    s0 = chk * CH + t4 * P
                    o_t = outs.tile([P, E], BF, tag="ot")
                    for ec in range(2):
                        ps_f = psA.tile([P, 512], F32, tag="ps")
                        for k2 in range(FT // 2):
                            nc.tensor.matmul(
                                ps_f,
                                gsw[:, 2 * k2:2 * k2 + 2, t4 * P:(t4 + 1) * P],
                                w3_sb[:, 2 * k2:2 * k2 + 2,
                                      ec * 512:(ec + 1) * 512],
                                start=(k2 == 0), stop=(k2 == FT // 2 - 1),
                                perf_mode=DR,
                            )
                        nc.vector.scalar_tensor_tensor(
                            out=o_t[:, ec * 512:(ec + 1) * 512], in0=ps_f,
                            scalar=1.0 / WS,
                            in1=xres[:, t4, ec * 512:(ec + 1) * 512],
                            op0=OP.mult, op1=OP.add,
                        )
                    nc.sync.dma_start(out=out_d.ap()[s0:s0 + P, :], in_=o_t)
    return nc


# --------------------------------------------------------------------------
def prep_core_inputs(inputs, core, seq=S):
    """Host-side data prep for one core."""
    b, hf = core // 2, core % 2
    hseq = seq // 2
    ET = E // P
    FT = 4 * E // P
    x = np.asarray(inputs["input_tensor"], np.float32)
    ln1_g = np.asarray(inputs["ln1_g"], np.float32)
    ln1_b = np.asarray(inputs["ln1_b"], np.float32)
    latents = np.asarray(inputs["latents"], np.float32)
    Wq = np.asarray(inputs["Wq"], np.float32)
    Wk = np.asarray(inputs["Wk"], np.float32)
    Wv = np.asarray(inputs["Wv"], np.float32)
    Wo = np.asarray(inputs["Wo"], np.float32)
    bo = np.asarray(inputs["bo"], np.float32)
    Wproj = np.asarray(inputs["Wproj"], np.float32)
    bproj = np.asarray(inputs["bproj"], np.float32)
    ln2_g = np.asarray(inputs["ln2_g"], np.float32)
    ln2_b = np.asarray(inputs["ln2_b"], np.float32)
    W1 = np.asarray(inputs["W1"], np.float32)
    b1 = np.asarray(inputs["b1"], np.float32)
    W2 = np.asarray(inputs["W2"], np.float32)
    b2 = np.asarray(inputs["b2"], np.float32)
    W3 = np.asarray(inputs["W3"], np.float32)
    b3 = np.asarray(inputs["b3"], np.float32)

    assert not (np.any(ln1_b) or np.any(ln2_b) or np.any(bo) or np.any(b1)
                or np.any(b2) or np.any(b3)), "nonzero biases unsupported"

    w1_dt = F8NP if W12_FP8 else BF16
    w1_s = WS if W12_FP8 else 1.0
    Wkf = np.transpose(Wk, (1, 0, 2)).reshape(E, H * HD)
    Wvf = np.transpose(Wv, (1, 0, 2)).reshape(E, H * HD)
    wk = (ln1_g[:, None] * Wkf * WS).astype(F8NP).reshape(ET, P, H * HD)
    wk = np.ascontiguousarray(wk.transpose(1, 0, 2))
    wv = (ln1_g[:, None] * Wvf * WS).astype(F8NP).reshape(ET, P, H * HD)
    wv = np.ascontiguousarray(wv.transpose(1, 0, 2))
    wq = Wq.astype(BF16).reshape(H, LD // P, P, HD).transpose(2, 0, 1, 3)
    wq = np.ascontiguousarray(wq).reshape(P, H * (LD // P) * HD)
    lat = latents.transpose(0, 2, 1).astype(BF16)              # [H, LD, NL]
    lat = lat.reshape(H, LD // P, P, NL).transpose(2, 0, 1, 3)
    lat = np.ascontiguousarray(lat).reshape(P, H * (LD // P) * NL)
    wo = np.ascontiguousarray(Wo.astype(BF16).reshape(H, HD, E).transpose(1, 0, 2))
    wproj = np.ascontiguousarray(Wproj[:, hf * hseq:(hf + 1) * hseq].astype(BF16))
    w1 = (ln2_g[:, None] * W1 * w1_s).astype(w1_dt).reshape(ET, P, FT, P)
    w1 = np.ascontiguousarray(w1.transpose(2, 1, 0, 3))
    w2 = (ln2_g[:, None] * W2).astype(BF16).reshape(ET, P, FT, P)
    w2 = np.ascontiguousarray(w2.transpose(2, 1, 0, 3))
    w3 = np.ascontiguousarray((W3 * WS).astype(F8NP).reshape(FT, P, E).transpose(1, 0, 2))
    x_full = np.ascontiguousarray(x[b, :seq]).astype(BF16)
    x_half = x[b, hf * hseq:(hf + 1) * hseq].copy()
    x_half += bproj[hf * hseq:(hf + 1) * hseq, None]
    return {
        "x_full": x_full, "x_half": np.ascontiguousarray(x_half.astype(BF16)),
        "wk": wk, "wv": wv, "wq": wq, "lat": lat, "wo": wo, "wproj": wproj,
        "w1": w1, "w2": w2, "w3": w3,
    }


_CACHE = {}


def kernel(**inputs) -> np.ndarray:
    if "nc" not in _CACHE:
        nc = bass.Bass("TRN2", target_bir_lowering=False, debug=False)
        build_program(nc, seq=S)
        _spill_extra_waits(nc)
        _CACHE["nc"] = nc
    nc = _CACHE["nc"]

    in_maps = [prep_core_inputs(inputs, c) for c in range(NCORES)]
    from concourse.bass_utils import run_bass_kernel_spmd
    res = run_bass_kernel_spmd(nc, in_maps, core_ids=list(range(NCORES)))

    out = np.empty((B, S, E), np.float32)
    for c in range(NCORES):
        b, hf = c // 2, c % 2
        out[b, hf * (S // 2):(hf + 1) * (S // 2)] = (
            res.results[c]["out"].astype(np.float32))
    return out
